# revision 63
# baseline (speedup 1.0000x reference)
"""Trainium2 Bass kernel: Conformer block (B=8, N=512, DIM=512, H=8, DH=64, FF=2048, CIN=1024, K=31).

Sharding: pure data-parallel over batch — each of the 8 NeuronCores processes one
batch item with the full weight set (no collectives).

Layout: activations are kept FEATURE-major ([feature, time] = x.T) on chip so that
chained matmuls need no transposes.  LayerNorm reductions over features are
ones-vector fp32r matmuls on the PE; the per-time-step affine factors live in
broadcast [128, N] tiles.

Precision strategy (tolerance 2e-2; measured ~9e-3):
  - The fat GEMMs (ff1/ff2 both matmuls, q/k/v projections, conv pointwise 1/2,
    and the depthwise conv) run in fp8e4m3 with DoubleRow perf mode: two
    128-deep contraction tiles per instruction, i.e. 2x the bf16 matmul rate
    and lower PE power (the PE power-throttle clamps sustained dense work to
    50% duty).  Weights are pre-scaled by 32/64 on the host so 0.02-scale
    values land mid-range in e4m3; the 1/alpha correction folds into the
    existing per-tile bias/scale ops (tensor_scalar with two scalar slots, or
    the activation scale operand).
  - Scores, attn@v, out-proj and the q@rel windows stay bf16; the rel shift
    transposes and LN statistics stay fp32r (a transpose must match the f32
    PSUM accumulation dtype bit-for-bit).
  - PSUM accumulation is fp32 everywhere; the residual stream is fp32.

The depthwise conv runs as 16 PSUM-accumulated DoubleRow diagonal matmuls per
128-channel block: taps k and k+16 pair up as the two 128-column halves of one
stationary tile, with the moving operand an overlapping [[542,2],[1,512]] view
of a glu tile stored twice (second copy shifted left 16 columns).  BN+swish
collapses into one Silu activation (scale=bn_s/32, bias=bn_t).

Relative-position attention uses the shift-gather trick: qr = q @ rel_emb.T is
bounced through an internal DRAM scratch and read back with a strided
(stride = row+1) access pattern so rel[j, i] = qr[i, i-j+512] lands directly as
the transposed score tile; softmax then runs over the partition axis with the
denominator via a ones-column fused into the attn@v matmul.
"""

import sys

for _p in ("/opt/trn_rl_repo", "/root/.axon_site/_ro/trn_rl_repo"):
    if _p not in sys.path:
        sys.path.insert(0, _p)

import numpy as np

B, N, DIM, H, DH, MULT, EXP, KW, MAXP = 8, 512, 512, 8, 64, 4, 2, 31, 512
INNER = H * DH
FF = DIM * MULT
CIN = DIM * EXP
EPS = 1e-5
P = 128
DT = DIM // P      # 4  feature tiles of the residual stream
FT = FF // P       # 16 ff hidden tiles
CT = CIN // P      # 8  conv channel tiles
NCORES = 8
PAD = KW - 1       # 30 causal pad
GW = PAD + N       # 542 glu tile width
NPAIR = 16         # depthwise taps (k, k+16), pair 15 has a zero second half

AW = 32.0          # fp8 weight scale (w1/w3, wq/wk/wv, c1, c2, dw)
AW2 = 64.0         # fp8 weight scale for w2/w4 (0.5 pre-folded)

# packed f32 constant tile column offsets ([P, CPK_W])
C_B1, C_B3 = 0, 16
C_BQ, C_BK, C_B2, C_B4, C_BO, C_C2B, C_PNG, C_PNB = 32, 36, 40, 44, 48, 52, 56, 60
C_C1A, C_C1G, C_BNS, C_BNT = 64, 72, 80, 88
CPK_W = 96


def build(split_waits=True):
    """Build the single-core Bass module (SPMD: same NEFF on all 8 cores)."""
    import concourse.bass as bass
    import concourse.mybir as mybir
    import concourse.tile as tile

    F32 = mybir.dt.float32
    F32R = mybir.dt.float32r
    BF16 = mybir.dt.bfloat16
    F8 = mybir.dt.float8e4
    AF = mybir.ActivationFunctionType
    AL = mybir.AluOpType
    DR = mybir.MatmulPerfMode.DoubleRow

    nc = bass.Bass()

    # ---------------- I/O ----------------
    xT_d = nc.dram_tensor("xT", [DIM, N], F32R, kind="ExternalInput")
    w1_d = nc.dram_tensor("w1", [2, P, 2 * FF], F8, kind="ExternalInput")
    w2_d = nc.dram_tensor("w2", [FT // 2, P, 2 * DIM], F8, kind="ExternalInput")
    wq_d = nc.dram_tensor("wq", [2, P, 2 * INNER], F8, kind="ExternalInput")
    wk_d = nc.dram_tensor("wk", [2, P, 2 * INNER], F8, kind="ExternalInput")
    wv_d = nc.dram_tensor("wv", [2, P, 2 * INNER], F8, kind="ExternalInput")
    bv_d = nc.dram_tensor("bvb", [P, INNER], F32, kind="ExternalInput")
    wo_d = nc.dram_tensor("wo", [2, P, 2 * DIM], F8, kind="ExternalInput")
    relT_d = nc.dram_tensor("relT", [P, 2 * MAXP + 1], BF16, kind="ExternalInput")
    c1_d = nc.dram_tensor("c1", [2, P, 4 * CIN], F8, kind="ExternalInput")
    dwd_d = nc.dram_tensor("dwdiag", [CT, P, NPAIR * 2 * P], F8, kind="ExternalInput")
    c2_d = nc.dram_tensor("c2", [CT // 2, P, 2 * DIM], F8, kind="ExternalInput")
    w3_d = nc.dram_tensor("w3", [2, P, 2 * FF], F8, kind="ExternalInput")
    w4_d = nc.dram_tensor("w4", [FT // 2, P, 2 * DIM], F8, kind="ExternalInput")
    cpk_d = nc.dram_tensor("cpk", [P, CPK_W], F32, kind="ExternalInput")
    antid_d = nc.dram_tensor("antid", [P, P], F32R, kind="ExternalInput")
    onesf_d = nc.dram_tensor("onesf", [P, P], F32R, kind="ExternalInput")

    outT_d = nc.dram_tensor("outT", [DIM, N], F32, kind="ExternalOutput")

    QRW = 2 * MAXP + 1  # 1025 relT row width

    with tile.TileContext(nc) as tc:
        with (
            nc.allow_low_precision(reason="fp8/bf16 matmul feeds"),
            tc.tile_pool(name="cst", bufs=1) as cst,
            tc.tile_pool(name="sb", bufs=2) as sb,
            tc.tile_pool(name="ps", bufs=2, space="PSUM") as psp,
        ):

            # ------- load x first (feeds the first LN-stat matmuls) -------
            xs = []
            for kt in range(DT):
                xt = sb.tile([P, N], F32R, tag="x", bufs=7)
                (nc.sync if kt % 2 == 0 else nc.scalar).dma_start(
                    xt[:, :], xT_d[kt * P:(kt + 1) * P, :])
                xs.append(xt)

            def full(t):
                return t if isinstance(t, bass.AP) else t[:, :]

            # ---------------- constants ----------------
            ones_full = cst.tile([P, P], F32R, tag="ones_full")
            nc.sync.dma_start(ones_full[:, :], onesf_d[:, :])
            cpk = cst.tile([P, CPK_W], F32, tag="cpk")
            nc.scalar.dma_start(cpk[:, :], cpk_d[:, :])
            identf = cst.tile([P, P], F32R, tag="ident")
            nc.scalar.dma_start(identf[:, :], antid_d[:, :])
            relT = cst.tile([P, QRW], BF16, tag="relT")
            nc.scalar.dma_start(relT[:, :], relT_d[:, :])
            bvt = cst.tile([P, INNER], F32, tag="bvt")
            nc.scalar.dma_start(bvt[:, :], bv_d[:, :])
            ones_bf = cst.tile([P, DH], BF16, tag="ones_bf")
            nc.vector.memset(ones_bf[:, :], 1.0)

            def two(ap, w):
                return ap.rearrange("p (two n) -> p two n", two=2)

            # ---------------- helpers ----------------
            def layer_norm_rc(xin, sq_eng=None, want_sub=True):
                """Stats of LN over the partition (feature) axis; the ones
                stationary is pre-scaled by 1/DIM so psums hold E[x], E[x^2].

                Returns (r_b, subs): z = subs[kt]*r_b, where subs = x - mean
                are computed off the mean PSUM while the rsqrt chain runs."""
                ps_sum = psp.tile([P, N], F32, tag="mm", bufs=2)
                for kt in range(DT):
                    nc.tensor.matmul(ps_sum[:, :], ones_full[:, :], full(xin[kt]),
                                     start=(kt == 0), stop=(kt == DT - 1))
                ps_sq = psp.tile([P, N], F32, tag="mm", bufs=2)
                for kt in range(DT):
                    xsq = sb.tile([P, N], F32R, tag="tmp", bufs=3)
                    if kt % 2 == 0:
                        nc.scalar.square(xsq[:, :], full(xin[kt]))
                    else:
                        nc.vector.tensor_mul(xsq[:, :], full(xin[kt]),
                                             full(xin[kt]))
                    nc.tensor.matmul(ps_sq[:, :], ones_full[:, :], xsq[:, :],
                                     start=(kt == 0), stop=(kt == DT - 1))
                msq = sb.tile([P, N], F32, tag="tmp", bufs=3)
                nc.scalar.square(msq[:, :], ps_sum[:, :])
                veps = sb.tile([P, N], F32, tag="tmp", bufs=3)
                nc.vector.scalar_tensor_tensor(veps[:, :], ps_sq[:, :], EPS,
                                               msq[:, :], AL.add, AL.subtract)
                lnv = sb.tile([P, N], F32, tag="tmp", bufs=3)
                nc.scalar.activation(lnv[:, :], veps[:, :], AF.Ln)
                subs = []
                if want_sub:
                    for kt in range(DT):
                        su = sb.tile([P, N], F32, tag="sub", bufs=4)
                        nc.vector.scalar_tensor_tensor(
                            su[:, :], full(xin[kt]), 1.0, ps_sum[:, :],
                            AL.mult, AL.subtract)
                        subs.append(su)
                r_b = sb.tile([P, N], F32, tag="r_b", bufs=2)
                nc.scalar.activation(r_b[:, :], lnv[:, :], AF.Exp, scale=-0.5)
                return r_b, subs

            def ln_apply8(r_b, subs):
                """LN apply into two fp8 supertiles [P, 2*N] (kt pairs)."""
                zsup = []
                for kp in range(2):
                    zt = sb.tile([P, 2 * N], F8, tag="z8", bufs=4)
                    for i in range(2):
                        kt = kp * 2 + i
                        nc.vector.tensor_mul(zt[:, i * N:(i + 1) * N],
                                             subs[kt][:, :], r_b[:, :])
                    zsup.append(zt)
                return zsup

            def ff_block(xin, w_d, bt_c, w2f_d, b2_c, sq_eng=None):
                """x + 0.5*ff(LN(x)); returns new residual tiles."""
                r_b, subs = layer_norm_rc(xin, sq_eng=sq_eng)
                zsup = ln_apply8(r_b, subs)
                # h = swish(z @ w1 + b1): fp8 DoubleRow, Silu on ACT
                h1sup = [sb.tile([P, 2 * N], F8, tag="h8", bufs=8,
                                 name=f"h1sup{i}") for i in range(FT // 2)]
                for half in range(2):
                    wts = []
                    for kp in range(2):
                        wt = sb.tile([P, 2 * (FF // 2)], F8, tag="wbig", bufs=5)
                        nc.sync.dma_start(
                            wt[:, :], w1_dram_slice(w_d, kp, half))
                        wts.append(wt)
                    for mh in range(FT // 2):
                        mt = half * (FT // 2) + mh
                        ph = psp.tile([P, N], F32, tag="acc", bufs=4)
                        for kp in range(2):
                            w3d = two(wts[kp][:, :], FF // 2)
                            nc.tensor.matmul(ph[:, :],
                                             w3d[:, :, mh * P:(mh + 1) * P],
                                             two(zsup[kp][:, :], N),
                                             perf_mode=DR,
                                             start=(kp == 0), stop=(kp == 1))
                        nc.scalar.activation(
                            h1sup[mt // 2][:, (mt % 2) * N:(mt % 2 + 1) * N],
                            ph[:, :], AF.Silu,
                            bias=cpk[:, bt_c + mt:bt_c + mt + 1], scale=1.0 / AW)
                # y = h @ w2: fp8 DoubleRow, mt-outer so each residual tile (and
                # the next block's LN) can start as soon as its column finishes
                wts2 = []
                for kq in range(FT // 4):
                    wt = sb.tile([P, 4 * DIM], F8, tag="wsmb", bufs=4)
                    nc.sync.dma_start(
                        wt[:, :].rearrange("p (k o) -> p k o", k=2),
                        bass.AP(w2f_d, kq * 2 * P * 2 * DIM,
                                [[2 * DIM, P], [P * 2 * DIM, 2], [1, 2 * DIM]]))
                    wts2.append(wt)
                xo = []
                for mt in range(DT):
                    py = psp.tile([P, N], F32, tag="acc", bufs=4)
                    for kp in range(FT // 2):
                        wt = wts2[kp // 2]
                        sub = kp % 2
                        w3d = two(wt[:, sub * 2 * DIM:(sub + 1) * 2 * DIM], DIM)
                        nc.tensor.matmul(py[:, :],
                                         w3d[:, :, mt * P:(mt + 1) * P],
                                         two(h1sup[kp][:, :], N),
                                         perf_mode=DR,
                                         start=(kp == 0), stop=(kp == FT // 2 - 1))
                    t1 = sb.tile([P, N], F32, tag="tmp", bufs=3)
                    nc.vector.tensor_scalar(out=t1[:, :], in0=py[:, :],
                                            scalar1=1.0 / AW2,
                                            scalar2=cpk[:, b2_c + mt:b2_c + mt + 1],
                                            op0=AL.mult, op1=AL.add)
                    t = sb.tile([P, N], F32R, tag="x", bufs=7)
                    nc.vector.tensor_add(t[:, :], t1[:, :], full(xin[mt]))
                    xo.append(t)
                return xo

            def w1_dram_slice(w_d, kp, half):
                return w_d[kp, :, half * FF:(half + 1) * FF]

            # ================= ff1 =================
            x1 = ff_block(xs, w1_d, C_B1, w2_d, C_B2, sq_eng=nc.scalar)

            # ================= attention =================
            r_b, subs = layer_norm_rc(x1)
            zsup = ln_apply8(r_b, subs)

            def proj_qk(w_dram, bias_c, tag):
                wts = []
                for kp in range(2):
                    wt = sb.tile([P, 2 * INNER], F8, tag="wsm", bufs=4)
                    nc.sync.dma_start(wt[:, :], w_dram[kp, :, :])
                    wts.append(wt)
                outs = []
                for mt in range(DT):
                    pq = psp.tile([P, N], F32, tag="mm", bufs=2)
                    for kp in range(2):
                        w3d = two(wts[kp][:, :], INNER)
                        nc.tensor.matmul(pq[:, :],
                                         w3d[:, :, mt * P:(mt + 1) * P],
                                         two(zsup[kp][:, :], N),
                                         perf_mode=DR,
                                         start=(kp == 0), stop=(kp == 1))
                    qt = sb.tile([P, N], BF16, tag=tag, bufs=4)
                    nc.vector.tensor_scalar(out=qt[:, :], in0=pq[:, :],
                                            scalar1=1.0 / AW,
                                            scalar2=cpk[:, bias_c + mt:bias_c + mt + 1],
                                            op0=AL.mult, op1=AL.add)
                    outs.append(qt)
                return outs

            qTs = proj_qk(wq_d, C_BQ, "qT")
            kTs = proj_qk(wk_d, C_BK, "kT")

            # v in time-major layout with a trailing ones column per head
            wvts = []
            for kp in range(2):
                wt = sb.tile([P, 2 * INNER], F8, tag="wsm", bufs=4)
                nc.sync.dma_start(wt[:, :], wv_d[kp, :, :])
                wvts.append(wt)
            vsup = [sb.tile([P, 2 * H * 2 * DH], F8, tag="vext", bufs=2,
                            name=f"vsup{i}") for i in range(2)]
            for jt in range(DT):
                pv = psp.tile([P, N], F32, tag="mm", bufs=2)
                for kp in range(2):
                    z3d = two(zsup[kp][:, :], N)
                    nc.tensor.matmul(pv[:, :],
                                     z3d[:, :, jt * P:(jt + 1) * P],
                                     two(wvts[kp][:, :], INNER),
                                     perf_mode=DR,
                                     start=(kp == 0), stop=(kp == 1))
                base = (jt % 2) * H * 2 * DH
                vw = vsup[jt // 2][:, base:base + H * 2 * DH].rearrange(
                    "p (h c) -> p h c", c=2 * DH)
                nc.vector.scalar_tensor_tensor(
                    vw[:, :, 0:DH],
                    pv[:, :].rearrange("p (h d) -> p h d", h=H), 1.0 / AW,
                    bvt[:, :].rearrange("p (h d) -> p h d", h=H),
                    AL.mult, AL.add)
                nc.vector.tensor_copy(
                    vw[:, :, DH:2 * DH],
                    ones_bf[:, None, :].broadcast_to([P, H, DH]))

            # scores (transposed), softmax over partitions, attn @ v.
            # qr = q @ relT is staged per (head, it) as a 640-wide SBUF window;
            # the shift-gather rel[j, i] = qr[i, 128 - i + j] is a per-partition
            # column shift, read directly by the transpose matmuls via a flat
            # diagonal access pattern (stride = row_pitch - 1).
            oTsup = [sb.tile([P, 2 * N], F8, tag="oT", bufs=2, name=f"oTsup{i}")
                     for i in range(2)]
            dmaq = [nc.gpsimd, nc.sync, nc.gpsimd, nc.sync]
            for h in range(H):
                hb = (h % 2) * DH
                qts = []
                for it in range(DT):
                    lq = qTs[h // 2][hb:hb + DH, it * P:(it + 1) * P]
                    cr0 = 3 * P - P * it
                    pq1 = psp.tile([P, 320], F32, tag="qrp", bufs=2)
                    nc.tensor.matmul(pq1[:, :], lq,
                                     relT[hb:hb + DH, cr0:cr0 + 320],
                                     start=True, stop=True)
                    pq2 = psp.tile([P, 320], F32, tag="qrp", bufs=2)
                    nc.tensor.matmul(pq2[:, :], lq,
                                     relT[hb:hb + DH, cr0 + 320:cr0 + 640],
                                     start=True, stop=True)
                    qt = sb.tile([P, 640], F32R, tag="qt", bufs=6)
                    nc.scalar.copy(qt[:, 0:320], pq1[:, :])
                    nc.vector.tensor_copy(qt[:, 320:640], pq2[:, :])
                    # shift-gather: rel[p, j] = qt[p, 128 - p + j], a flat
                    # diagonal SBUF->SBUF DMA (per-partition column shift)
                    qf = qt[:, :]
                    rel = sb.tile([P, N], F32R, tag="rel", bufs=6)
                    dmaq[it].dma_start(
                        rel[:, :],
                        bass.AP(qf.tensor, qf.offset + P,
                                [[qf.ap[0][0] - 1, P], [1, N]]))
                    qts.append(rel)
                esup = [sb.tile([P, 2 * N], F8, tag="exp", bufs=4,
                                name=f"esup{i}") for i in range(2)]
                for jt in range(DT):
                    pss = psp.tile([P, N], F32, tag="acc", bufs=4)
                    nc.tensor.matmul(pss[:, :],
                                     kTs[h // 2][hb:hb + DH, jt * P:(jt + 1) * P],
                                     qTs[h // 2][hb:hb + DH, :],
                                     start=True, stop=False)
                    for it in range(DT):
                        nc.tensor.matmul(
                            pss[:, it * P:(it + 1) * P].bitcast(F32R),
                            qts[it][:, jt * P:(jt + 1) * P], identf[:, :],
                            is_transpose=True, start=False, stop=(it == DT - 1))
                    nc.scalar.activation(
                        esup[jt // 2][:, (jt % 2) * N:(jt % 2 + 1) * N],
                        pss[:, :], AF.Exp)
                po = psp.tile([P, N], F32, tag="mm", bufs=2)
                for jp in range(2):
                    v3d = two(vsup[jp][:, :], H * 2 * DH)
                    nc.tensor.matmul(po[:, :],
                                     v3d[:, :, h * 2 * DH:(h + 1) * 2 * DH],
                                     two(esup[jp][:, :], N),
                                     perf_mode=DR,
                                     start=(jp == 0), stop=(jp == 1))
                lnd = sb.tile([DH, N], F32, tag="dwt", bufs=3)
                nc.scalar.activation(lnd[:, :], po[DH:2 * DH, :], AF.Ln)
                rb = sb.tile([DH, N], F32, tag="dwt", bufs=3)
                nc.scalar.activation(rb[:, :], lnd[:, :], AF.Exp, scale=-1.0)
                ob = (h // 2) % 2
                nc.vector.tensor_mul(
                    oTsup[h // 4][hb:hb + DH, ob * N:(ob + 1) * N],
                    po[0:DH, :], rb[:, :])

            # out-projection + residual (fp8 DoubleRow)
            wots = []
            for kp in range(2):
                wt = sb.tile([P, 2 * DIM], F8, tag="wsm2", bufs=4)
                nc.sync.dma_start(wt[:, :], wo_d[kp, :, :])
                wots.append(wt)
            x2 = []
            x2sup = [sb.tile([P, 2 * N], F8, tag="xb8", bufs=2, name=f"x2sup{i}")
                     for i in range(2)]
            for mt in range(DT):
                pa2 = psp.tile([P, N], F32, tag="acc", bufs=4)
                for kp in range(2):
                    w3d = two(wots[kp][:, :], DIM)
                    nc.tensor.matmul(pa2[:, :],
                                     w3d[:, :, mt * P:(mt + 1) * P],
                                     two(oTsup[kp][:, :], N),
                                     perf_mode=DR,
                                     start=(kp == 0), stop=(kp == 1))
                t1 = sb.tile([P, N], F32, tag="tmp", bufs=3)
                nc.vector.tensor_scalar(out=t1[:, :], in0=pa2[:, :],
                                        scalar1=1.0 / AW,
                                        scalar2=cpk[:, C_BO + mt:C_BO + mt + 1],
                                        op0=AL.mult, op1=AL.add)
                t = sb.tile([P, N], F32R, tag="x", bufs=7)
                nc.vector.tensor_add(t[:, :], t1[:, :], x1[mt][:, :])
                x2.append(t)
                nc.vector.tensor_copy(
                    x2sup[mt // 2][:, (mt % 2) * N:(mt % 2 + 1) * N], t[:, :])

            # ================= conv module =================
            # glu tiles stored twice: [0:GW] plain, [GW:2*GW-16] shifted left 16.
            # The cool (mostly-zero stationary) depthwise DoubleRow matmuls are
            # interleaved per channel block between the hot conv1 GEMMs to keep
            # average PE power below the 50%-duty throttle threshold.
            hcsup = [sb.tile([P, 2 * N], F8, tag="hc8", bufs=2, name=f"hcsup{i}")
                     for i in range(CT // 2)]
            glus = []
            for half in range(2):
                c1ts = []
                for kp in range(2):
                    wt = sb.tile([P, 2 * CIN], F8, tag="wbig", bufs=5)
                    nc.sync.dma_start(wt[:, :],
                                      c1_d[kp, :, half * CIN * 2:(half + 1) * CIN * 2])
                    c1ts.append(wt)
                for ch in range(CT // 2):
                    ct = half * (CT // 2) + ch
                    pa = psp.tile([P, N], F32, tag="acc", bufs=4)
                    pg = psp.tile([P, N], F32, tag="acc", bufs=4)
                    for kp in range(2):
                        w3d = two(c1ts[kp][:, :], CIN)
                        nc.tensor.matmul(pa[:, :],
                                         w3d[:, :, ch * P:ch * P + P],
                                         two(x2sup[kp][:, :], N),
                                         perf_mode=DR,
                                         start=(kp == 0), stop=(kp == 1))
                    for kp in range(2):
                        w3d = two(c1ts[kp][:, :], CIN)
                        nc.tensor.matmul(pg[:, :],
                                         w3d[:, :, (CT // 2 + ch) * P:
                                             (CT // 2 + ch) * P + P],
                                         two(x2sup[kp][:, :], N),
                                         perf_mode=DR,
                                         start=(kp == 0), stop=(kp == 1))
                    sig = sb.tile([P, N], F32, tag="csig", bufs=4)
                    nc.scalar.activation(sig[:, :], pg[:, :], AF.Sigmoid,
                                         bias=cpk[:, C_C1G + ct:C_C1G + ct + 1],
                                         scale=1.0 / AW)
                    gt = sb.tile([P, N], F32, tag="cgt", bufs=4)
                    nc.vector.tensor_scalar(out=gt[:, :], in0=pa[:, :],
                                            scalar1=1.0 / AW,
                                            scalar2=cpk[:, C_C1A + ct:C_C1A + ct + 1],
                                            op0=AL.mult, op1=AL.add)
                    glu = sb.tile([P, 2 * GW], F8, tag="glu", bufs=4)
                    nc.vector.memset(glu[:, 0:PAD], 0.0)
                    nc.vector.memset(glu[:, GW:GW + PAD - 16], 0.0)
                    nc.vector.memset(glu[:, 2 * GW - 16:2 * GW], 0.0)
                    nc.vector.tensor_mul(glu[:, PAD:GW], gt[:, :], sig[:, :])
                    nc.gpsimd.tensor_mul(glu[:, GW + PAD - 16:2 * GW - 16],
                                         gt[:, :], sig[:, :])
                    glus.append(glu)

            # depthwise conv: 16 DoubleRow diagonal-pair matmuls per block
            for ct in range(CT):
                dg = sb.tile([P, NPAIR * 2 * P], F8, tag="dg", bufs=2)
                nc.sync.dma_start(dg[:, :], dwd_d[ct, :, :])
                pd = psp.tile([P, N], F32, tag="mm", bufs=2)
                gfull = glus[ct][:, :]
                gpitch = gfull.ap[0][0]
                for i in range(NPAIR):
                    rhs = bass.AP(gfull.tensor, gfull.offset + i,
                                  [[gpitch, P], [GW, 2], [1, N]])
                    nc.tensor.matmul(pd[:, :],
                                     two(dg[:, i * 2 * P:(i + 1) * 2 * P], P),
                                     rhs,
                                     perf_mode=DR,
                                     start=(i == 0), stop=(i == NPAIR - 1))
                nc.scalar.activation(
                    hcsup[ct // 2][:, (ct % 2) * N:(ct % 2 + 1) * N],
                    pd[:, :], AF.Silu,
                    bias=cpk[:, C_BNT + ct:C_BNT + ct + 1],
                    scale=cpk[:, C_BNS + ct:C_BNS + ct + 1])

            # conv2 + residual (mt-outer for early LN3 start)
            c2ts = []
            for kp in range(CT // 2):
                wt = sb.tile([P, 2 * DIM], F8, tag="wsm", bufs=4)
                nc.sync.dma_start(wt[:, :], c2_d[kp, :, :])
                c2ts.append(wt)
            x3 = []
            for mt in range(DT):
                pc = psp.tile([P, N], F32, tag="acc", bufs=4)
                for kp in range(CT // 2):
                    w3d = two(c2ts[kp][:, :], DIM)
                    nc.tensor.matmul(pc[:, :],
                                     w3d[:, :, mt * P:(mt + 1) * P],
                                     two(hcsup[kp][:, :], N),
                                     perf_mode=DR,
                                     start=(kp == 0), stop=(kp == CT // 2 - 1))
                t1 = sb.tile([P, N], F32, tag="tmp", bufs=3)
                nc.vector.tensor_scalar(out=t1[:, :], in0=pc[:, :],
                                        scalar1=1.0 / AW,
                                        scalar2=cpk[:, C_C2B + mt:C_C2B + mt + 1],
                                        op0=AL.mult, op1=AL.add)
                t = sb.tile([P, N], F32R, tag="x", bufs=7)
                nc.vector.tensor_add(t[:, :], t1[:, :], x2[mt][:, :])
                x3.append(t)

            # ================= ff2 =================
            x4 = ff_block(x3, w3_d, C_B3, w4_d, C_B4)

            # ================= post-LN =================
            # out = ((x - m)*r)*g + b: subs overlap the rsqrt chain, then only
            # 2 dependent DVE ops per tile remain
            r_b, subs = layer_norm_rc(x4)
            for mt in range(DT):
                eng = nc.vector if mt < 3 else nc.gpsimd
                t = sb.tile([P, N], F32, tag="lnt", bufs=4)
                eng.tensor_mul(t[:, :], subs[mt][:, :], r_b[:, :])
                ot = sb.tile([P, N], F32, tag="outt", bufs=4)
                eng.tensor_scalar(out=ot[:, :], in0=t[:, :],
                                  scalar1=cpk[:, C_PNG + mt:C_PNG + mt + 1],
                                  scalar2=cpk[:, C_PNB + mt:C_PNB + mt + 1],
                                  op0=AL.mult, op1=AL.add)
                (nc.sync if mt % 2 == 0 else nc.scalar).dma_start(
                    outT_d[mt * P:(mt + 1) * P, :], ot[:, :])

    if split_waits:
        _split_matmul_waits(nc, mybir)
    return nc


def _split_matmul_waits(nc, mybir):
    """This walrus build rejects engine instructions carrying more than one
    sync wait; hoist the extras onto EventSemaphore instructions on the same
    engine queue right before the instruction."""
    fn = nc.m.functions[0]
    ctr = 0
    for blk in fn.blocks:
        out = []
        changed = False
        for ins in blk.instructions:
            si = ins.sync_info
            if (si is not None and si.on_wait and len(si.on_wait) > 1
                    and not isinstance(ins, (mybir.InstEventSemaphore,
                                             mybir.InstNoOp))):
                waits = list(si.on_wait)
                for w in waits[:-1]:
                    ev = mybir.InstNoOp(
                        name=f"EVW-{ctr}", ins=[], outs=[],
                        sync_info=mybir.SyncInfo(on_wait=[w], on_update=[]))
                    ev.engine = ins.engine
                    ctr += 1
                    out.append(ev)
                ins.sync_info = mybir.SyncInfo(
                    on_wait=[waits[-1]], on_update=list(si.on_update or []))
                changed = True
            out.append(ins)
        if changed:
            blk.instructions = out


def _dr_first(w, alpha, f8):
    """[K, O] -> [K/256, P, 2*O] fp8: k = kp*256 + khalf*128 + p."""
    K, O = w.shape
    r = (w * alpha).reshape(K // 256, 2, P, O).transpose(0, 2, 1, 3)
    return np.ascontiguousarray(r.reshape(K // 256, P, 2 * O).astype(f8))


def prep_inputs(inputs):
    """Host-side preprocessing: fold LN affines / scales / biases into weights."""
    import ml_dtypes

    f = np.float32
    bf = ml_dtypes.bfloat16
    f8 = ml_dtypes.float8_e4m3
    ii = {k: np.asarray(v, dtype=f) for k, v in inputs.items()}

    def colmaj(b, nb):
        return np.ascontiguousarray(b.astype(f).reshape(nb, P).T)

    g1, be1 = ii["ff1_ln_g"], ii["ff1_ln_b"]
    # w1 fp8 layout: [kp, p, half*2048 + khalf*1024 + o]
    def ff_first(wfull, g):
        ws = (g[:, None] * wfull * AW).reshape(2, 2, P, 2, FF // 2)
        ws = ws.transpose(0, 2, 3, 1, 4)  # kp, p, half, khalf, o
        return np.ascontiguousarray(ws.reshape(2, P, 2 * FF).astype(f8))

    w1 = ff_first(ii["ff1_w1"], g1)
    b1 = colmaj(be1 @ ii["ff1_w1"] + ii["ff1_b1"], FT)
    w2 = _dr_first(0.5 * ii["ff1_w2"], AW2, f8)
    b2 = colmaj(0.5 * ii["ff1_b2"], DT)

    ag, ab = ii["attn_ln_g"], ii["attn_ln_b"]
    sc = DH ** -0.5
    wq = _dr_first(ag[:, None] * ii["wq"] * sc, AW, f8)
    bq = colmaj((ab @ ii["wq"] + ii["bq"]) * sc, DT)
    wkv, bkv = ii["wkv"], ii["bkv"]
    wk = _dr_first(ag[:, None] * wkv[:, :INNER], AW, f8)
    bk = colmaj(ab @ wkv[:, :INNER] + bkv[:INNER], DT)
    wv = _dr_first(ag[:, None] * wkv[:, INNER:], AW, f8)
    bv = np.ascontiguousarray(np.broadcast_to(
        ab @ wkv[:, INNER:] + bkv[INNER:], (P, INNER)))
    wo = _dr_first(ii["wo"], AW, f8)
    bo = colmaj(ii["bo"], DT)
    # relT rows: head feature d lives at partition (h%2)*64 + d -> duplicate rows
    rT = ii["rel_emb"].T[:, ::-1]  # [64, 1025] column-reversed
    relT = np.ascontiguousarray(np.concatenate([rT, rT], axis=0).astype(bf))

    # c1 columns reordered to match the kernel's half-split loop:
    # half h covers channel blocks ct=4h..4h+3 and lays out [a-cols | g-cols]
    w = ii["conv1_w"]
    HC = CIN // 2  # 512
    c1re = np.concatenate(
        [w[:, 0:HC], w[:, CIN:CIN + HC], w[:, HC:CIN], w[:, CIN + HC:]], axis=1)
    c1 = ff_first(c1re, np.ones(DIM, f))  # same [kp, p, half, khalf, 1024] layout
    c1b = ii["conv1_b"]
    c1a = colmaj(c1b[:CIN], CT)
    c1g = colmaj(c1b[CIN:], CT)
    # DoubleRow diagonal pairs: pair i = taps (i, i+16); tap 31 = zeros
    dwd = np.zeros((CT, P, NPAIR, 2, P), dtype=f8)
    wr = (ii["dw_w"] * AW).reshape(CT, P, KW)
    pp = np.arange(P)
    for ct in range(CT):
        for i in range(NPAIR):
            dwd[ct, pp, i, 0, pp] = wr[ct, :, i].astype(f8)
            if i + 16 < KW:
                dwd[ct, pp, i, 1, pp] = wr[ct, :, i + 16].astype(f8)
    dwd = np.ascontiguousarray(dwd.reshape(CT, P, NPAIR * 2 * P))
    inv = 1.0 / np.sqrt(ii["bn_var"] + EPS)
    s = inv * ii["bn_g"]
    t = ii["bn_b"] - ii["bn_mean"] * s
    bns = colmaj(s / AW, CT)
    bnt = colmaj(t + s * ii["dw_b"], CT)
    c2 = _dr_first(ii["conv2_w"], AW, f8)
    c2b = colmaj(ii["conv2_b"], DT)

    g3, be3 = ii["ff2_ln_g"], ii["ff2_ln_b"]
    w3 = ff_first(ii["ff2_w1"], g3)
    b3 = colmaj(be3 @ ii["ff2_w1"] + ii["ff2_b1"], FT)
    w4 = _dr_first(0.5 * ii["ff2_w2"], AW2, f8)
    b4 = colmaj(0.5 * ii["ff2_b2"], DT)

    png = colmaj(ii["pn_g"], DT)
    pnb = colmaj(ii["pn_b"], DT)

    cpk = np.zeros((P, CPK_W), dtype=f)
    cpk[:, C_B1:C_B1 + FT] = b1
    cpk[:, C_B3:C_B3 + FT] = b3
    cpk[:, C_BQ:C_BQ + DT] = bq
    cpk[:, C_BK:C_BK + DT] = bk
    cpk[:, C_B2:C_B2 + DT] = b2
    cpk[:, C_B4:C_B4 + DT] = b4
    cpk[:, C_BO:C_BO + DT] = bo
    cpk[:, C_C2B:C_C2B + DT] = c2b
    cpk[:, C_PNG:C_PNG + DT] = png
    cpk[:, C_PNB:C_PNB + DT] = pnb
    cpk[:, C_C1A:C_C1A + CT] = c1a
    cpk[:, C_C1G:C_C1G + CT] = c1g
    cpk[:, C_BNS:C_BNS + CT] = bns
    cpk[:, C_BNT:C_BNT + CT] = bnt

    shared = dict(w1=w1, w2=w2, wq=wq, wk=wk, wv=wv, bvb=bv, wo=wo, relT=relT,
                  c1=c1, dwdiag=dwd, c2=c2, w3=w3, w4=w4, cpk=cpk,
                  antid=np.ascontiguousarray(np.eye(P, dtype=f)),
                  onesf=np.full((P, P), 1.0 / DIM, dtype=f))
    x = ii["x"]
    in_maps = []
    for b in range(NCORES):
        m = dict(shared)
        m["xT"] = np.ascontiguousarray(x[b].T)
        in_maps.append(m)
    return in_maps


_BUILT = None


def run(inputs, trace=False):
    global _BUILT
    from concourse import bass_utils

    in_maps = prep_inputs(inputs)
    if _BUILT is None:
        _BUILT = build()
    res = bass_utils.run_bass_kernel_spmd(
        _BUILT, in_maps, core_ids=list(range(NCORES)), trace=trace)
    out = np.stack([np.asarray(r["outT"]).T for r in res.results])
    return np.ascontiguousarray(out.astype(np.float32)), res


def kernel(**inputs):
    out, _ = run(inputs, trace=False)
    return out
